# revision 21
# baseline (speedup 1.0000x reference)
"""AttnGreedySearchV2 Trainium2 kernel.

Math (per batch row):
  corpus = item @ W_proj + b_proj          [10, 16]
  t_vec  = tanh(corpus @ W_t)              [10, 16]
  S = u; for k in 0..sn-1:
      s = tanh((S / (k+1)) @ W_s)          [16]
      c* = argmax_c <t_vec[c], s>
      pick v_k = corpus[c*]; S += v_k
  out = [u, v_0..v_{sn-1}]                 [1+sn, 16]

Device layout (per core, R=8192 rows, r = b_lo*1024 + b_hi):
  layout C:  partition p = 16*b_lo + h, free col = b_hi           (corpus, S, s)
  layout B': partition q, free (k, ...) with b_hi = 8*q + k       (t_vec, scores)
Scores/argmax run in B' (h on free dim -> DVE segmented reduce); the argmax
index is moved into ap_gather's per-core wrapped format with one PE transpose
plus two constant permutation matmuls.
"""

import numpy as np
from contextlib import ExitStack

import concourse.bass as bass
import concourse.bacc as bacc
import concourse.tile as tile
from concourse import mybir
from concourse.bass_utils import run_bass_kernel_spmd

F32 = mybir.dt.float32
I16 = mybir.dt.int16
U8 = mybir.dt.uint8

NCORES = 8
BS = 65536
R = BS // NCORES          # 8192 rows per core
NB = R // 8               # 1024 (b_hi)
CORPUS = 10
HID = 16
IND = 100
NELEM = NB * CORPUS       # 10240 free cols in layout C (b_hi, c)

_AFT = mybir.ActivationFunctionType


def _blockdiag(w):
    out = np.zeros((128, 128), dtype=np.float32)
    for b in range(8):
        out[16 * b:16 * b + 16, 16 * b:16 * b + 16] = w
    return out


def build_consts(W_proj, b_proj, W_s, W_t, sn):
    # wproj: [101, 8*128]; block b at cols [128b:128b+128) holds W_eff in
    # cols [16b:16b+16) of that block; row 100 is the bias row.
    wproj = np.zeros((101, 8 * 128), dtype=np.float32)
    for b in range(8):
        wproj[0:100, 128 * b + 16 * b:128 * b + 16 * b + 16] = W_proj
        wproj[100, 128 * b + 16 * b:128 * b + 16 * b + 16] = b_proj
    wt = _blockdiag(W_t)
    ws = np.zeros((128, sn * 128), dtype=np.float32)
    for k in range(sn):
        ws[:, 128 * k:128 * (k + 1)] = _blockdiag(W_s / float(k + 1))
    ident = np.eye(128, dtype=np.float32)
    # per-stripe perm: P2[par][f=8*kl+b_lo, 16*b_lo + 2*par + kl] = 1, par=q%8
    perm = np.zeros((16, 8 * 128), dtype=np.float32)
    for kl in range(2):
        for b_lo in range(8):
            for par in range(8):
                perm[8 * kl + b_lo, 128 * par + 16 * b_lo + 2 * par + kl] = 1.0
    # offs[p, s] = 10*(16*s + p%16): local elem index base within a stripe
    p = np.arange(128)[:, None]
    s = np.arange(16)[None, :]
    offs = (10.0 * (16 * s + (p % 16))).astype(np.float32)   # [128, 16]
    iotac = np.tile(np.arange(CORPUS, dtype=np.float32), (128, 1))
    big = np.full((128, 1), 1e9, dtype=np.float32)
    return dict(wproj=wproj, wt=wt, ws=ws, ident=ident, perm=perm,
                offs=offs, iotac=iotac, big=big)


def build_nc(sn, debug=False):
    nc = bacc.Bacc(None, target_bir_lowering=False)
    itemT = nc.declare_dram_parameter("itemT", [8, 101, NELEM], F32, isOutput=False)
    uT = nc.declare_dram_parameter("uT", [128, NB], F32, isOutput=False)
    wproj_d = nc.declare_dram_parameter("wproj", [101, 8 * 128], F32, isOutput=False)
    wt_d = nc.declare_dram_parameter("wt", [128, 128], F32, isOutput=False)
    ws_d = nc.declare_dram_parameter("ws", [128, sn * 128], F32, isOutput=False)
    ident_d = nc.declare_dram_parameter("ident", [128, 128], F32, isOutput=False)
    perm_d = nc.declare_dram_parameter("perm", [16, 8 * 128], F32, isOutput=False)
    offs_d = nc.declare_dram_parameter("offs", [128, 16], F32, isOutput=False)
    iotac_d = nc.declare_dram_parameter("iotac", [128, CORPUS], F32, isOutput=False)
    big_d = nc.declare_dram_parameter("big", [128, 1], F32, isOutput=False)
    out_d = nc.declare_dram_parameter("out", [128, sn * NB], F32, isOutput=True)
    if debug:
        dbg_corpus = nc.declare_dram_parameter("dbg_corpus", [128, NELEM], F32, isOutput=True)
        dbg_tB = nc.declare_dram_parameter("dbg_tB", [128, NELEM], F32, isOutput=True)
        dbg_sC = nc.declare_dram_parameter("dbg_sC", [128, NB], F32, isOutput=True)
        dbg_sB = nc.declare_dram_parameter("dbg_sB", [128, NB], F32, isOutput=True)
        dbg_scores = nc.declare_dram_parameter("dbg_scores", [128, 640], F32, isOutput=True)
        dbg_idxB = nc.declare_dram_parameter("dbg_idxB", [128, 64], F32, isOutput=True)
        dbg_idxs = nc.declare_dram_parameter("dbg_idxs", [128, NB // 16], I16, isOutput=True)

    with tile.TileContext(nc) as tc, ExitStack() as ctx:
        cpool = ctx.enter_context(tc.tile_pool(name="consts", bufs=1))
        cwproj = cpool.tile([101, 8 * 128], F32, tag="cwproj")
        nc.sync.dma_start(cwproj[:], wproj_d[:])
        cwt = cpool.tile([128, 128], F32, tag="cwt")
        nc.sync.dma_start(cwt[:], wt_d[:])
        cws = cpool.tile([128, sn * 128], F32, tag="cws")
        nc.sync.dma_start(cws[:], ws_d[:])
        cid = cpool.tile([128, 128], F32, tag="cid")
        nc.sync.dma_start(cid[:], ident_d[:])
        cperm = cpool.tile([16, 8 * 128], F32, tag="cperm")
        nc.sync.dma_start(cperm[:], perm_d[:])
        coffs = cpool.tile([128, 16], F32, tag="coffs")
        nc.sync.dma_start(coffs[:], offs_d[:])
        ciota = cpool.tile([128, CORPUS], F32, tag="ciota")
        nc.sync.dma_start(ciota[:], iotac_d[:])
        cbig = cpool.tile([128, 1], F32, tag="cbig")
        nc.sync.dma_start(cbig[:], big_d[:])

        # PE warm-up touches: make the PE observe each const's DMA lane via
        # tiny 1x1 matmuls so real matmuls carry at most one sync wait
        # (walrus S3_LW struct limit for fp32 matmuls).
        def pe_touch(tgt, src_ap=None):
            a = (src_ap if src_ap is not None else cid)[0:1, 0:1]
            nc.tensor.matmul(tgt[0:1, 0:1], lhsT=a, rhs=a, start=True, stop=True)
        with tc.tile_pool(name="warmps", bufs=1, space="PSUM") as wps_pool:
            wtile = wps_pool.tile([1, 1], F32, tag="wt")
            for cst in (cwproj, cwt, cid, cws, cperm):
                pe_touch(wtile, cst)

        main = ctx.enter_context(tc.tile_pool(name="main", bufs=1))
        corpus = main.tile([128, NELEM], F32, tag="corpus")
        tB = main.tile([128, NELEM], F32, tag="tB")
        outitems = main.tile([128, sn * NB], F32, tag="outitems")
        S = main.tile([128, NB], F32, tag="S")
        nc.sync.dma_start(S[:], uT[:])

        # iteration-0 s-stage depends only on S=u: emit before the projection
        sB0 = {}
        with tc.tile_pool(name="hpsum", bufs=1, space="PSUM") as hps:
            for g in range(4):
                hsps = hps.tile([128, 256], F32, tag="hsps")
                pe_touch(hsps)
                nc.tensor.matmul(hsps[:], lhsT=cws[:, 0:128],
                                 rhs=S[:, 256 * g:256 * (g + 1)],
                                 start=True, stop=True)
                hsC = main.tile([128, 256], F32, tag=f"hsC{g}")
                nc.scalar.activation(hsC[:], hsps[:], _AFT.Tanh)
                hsbp = hps.tile([128, 256], F32, tag="hsbp")
                hsC_v = hsC[:].rearrange("p (q kk) -> p kk q", kk=2)
                for kl in range(2):
                    nc.tensor.transpose(hsbp[:, 128 * kl:128 * (kl + 1)],
                                        hsC_v[:, kl, :], cid[:])
                sB0g = main.tile([128, 256], F32, tag=f"hsB{g}")
                nc.scalar.activation(sB0g[:], hsbp[:], _AFT.Copy)
                sB0[g] = sB0g
        # ------- projection + t-phase, interleaved per stripe -------
        W = 256
        itemT_v = itemT[:].rearrange("b d n -> d b n")
        qchunks = [(0, 51), (51, 51), (102, 26)]
        with tc.tile_pool(name="proj", bufs=2) as ppool, \
             tc.tile_pool(name="ppsum", bufs=2, space="PSUM") as pps, \
             tc.tile_pool(name="tch", bufs=2) as tchp, \
             tc.tile_pool(name="tpsum", bufs=1, space="PSUM") as tps_pool, \
             tc.tile_pool(name="tbpsum", bufs=1, space="PSUM") as tbp_pool:
            for g in range(4):
                for wl in range(10):
                    w = g * 10 + wl
                    it = ppool.tile([101, 8 * W], F32, tag="it")
                    itv = it[:].rearrange("d (b n) -> d b n", b=8)
                    nc.sync.dma_start(itv, itemT_v[:, :, w * W:(w + 1) * W])
                    ps = pps.tile([128, W], F32, tag="ps")
                    pe_touch(ps)
                    for b in range(8):
                        nc.tensor.matmul(ps[:], lhsT=cwproj[:, 128 * b:128 * b + 128],
                                         rhs=it[:, b * W:(b + 1) * W],
                                         start=(b == 0), stop=(b == 7))
                    nc.scalar.activation(corpus[:, w * W:(w + 1) * W], ps[:], _AFT.Copy)
                for klw in range(2):
                    k = 2 * g + klw
                    blk = corpus[:, g * 2560:(g + 1) * 2560]
                    corpus_vk = blk.rearrange("p (q kk c) -> p kk q c", kk=2, c=CORPUS)
                    tps = tps_pool.tile([128, 1536], F32, tag="tps")
                    pe_touch(tps)
                    for ci, (q0, qn) in enumerate(qchunks):
                        rhs = corpus_vk[:, klw, q0:q0 + qn, :]
                        nc.tensor.matmul(tps[:, 512 * ci:512 * ci + qn * 10], lhsT=cwt[:],
                                         rhs=rhs, start=True, stop=True)
                    tch = tchp.tile([128, 1280], F32, tag="tch")
                    for ci, (q0, qn) in enumerate(qchunks):
                        nc.scalar.activation(tch[:, q0 * 10:(q0 + qn) * 10],
                                             tps[:, 512 * ci:512 * ci + qn * 10], _AFT.Tanh)
                    tbp = tbp_pool.tile([128, 1280], F32, tag="tbp")
                    tch_v = tch[:].rearrange("p (q c) -> p c q", c=CORPUS)
                    for c in range(CORPUS):
                        nc.tensor.transpose(tbp[:, 128 * c:128 * (c + 1)],
                                            tch_v[:, c, :], cid[:])
                    nc.scalar.activation(tB[:, 1280 * k:1280 * (k + 1)], tbp[:], _AFT.Copy)

        # ---------------- greedy search loop (4 stripes of 2 k-windows) ----
        with tc.tile_pool(name="loop", bufs=2) as lp, \
             tc.tile_pool(name="prp", bufs=2) as prp, \
             tc.tile_pool(name="spsum", bufs=2, space="PSUM") as sps_pool, \
             tc.tile_pool(name="ipsum", bufs=2, space="PSUM") as ips_pool:
            def s_stage(ki, g):
                # s = tanh(BD(W_s/(ki+1)) @ S_g)
                sps = sps_pool.tile([128, 256], F32, tag="sps")
                pe_touch(sps)
                nc.tensor.matmul(sps[:], lhsT=cws[:, 128 * ki:128 * (ki + 1)],
                                 rhs=S[:, 256 * g:256 * (g + 1)],
                                 start=True, stop=True)
                sC = lp.tile([128, 256], F32, tag=f"sC{g}")
                nc.scalar.activation(sC[:], sps[:], _AFT.Tanh)
                sbp = sps_pool.tile([128, 256], F32, tag="sbp")
                sC_v = sC[:].rearrange("p (q kk) -> p kk q", kk=2)
                for kl in range(2):
                    nc.tensor.transpose(sbp[:, 128 * kl:128 * (kl + 1)],
                                        sC_v[:, kl, :], cid[:])
                sB = lp.tile([128, 256], F32, tag=f"sB{g}")
                nc.scalar.activation(sB[:], sbp[:], _AFT.Copy)
                return sB
            for ki in range(sn):
                for g in range(4):
                    sB = sB0[g] if ki == 0 else s_stage(ki, g)

                    # scores for k in {2g, 2g+1}, one chunk per k-window
                    scores = lp.tile([128, 160], F32, tag=f"scores{g}")
                    for kl in range(2):
                        pr = prp.tile([128, 1280], F32, tag=f"pr{g}")
                        t_in = tB[:, (2 * g + kl) * 1280:(2 * g + kl + 1) * 1280].rearrange(
                            "p (c bh) -> p c bh", c=CORPUS)
                        s_in = sB[:, 128 * kl:128 * (kl + 1)].unsqueeze(1).broadcast_to(
                            [128, CORPUS, 128])
                        pr_v = pr[:].rearrange("p (c bh) -> p c bh", c=CORPUS)
                        nc.vector.tensor_mul(pr_v, t_in, s_in)
                        red_in = pr[:].rearrange("p (a h) -> p a h", h=HID)
                        nc.vector.reduce_sum(scores[:, 80 * kl:80 * (kl + 1)],
                                             red_in, axis=mybir.AxisListType.X)

                    # argmax over c (first max)
                    sc_v = scores[:].rearrange("p (kk c b) -> p kk c b",
                                               kk=2, c=CORPUS).transpose([0, 1, 3, 2])
                    smax = lp.tile([128, 16], F32, tag=f"smax{g}")
                    smax_v = smax[:].rearrange("p (kk b) -> p kk b", kk=2)
                    nc.vector.reduce_max(smax_v, sc_v, axis=mybir.AxisListType.X)
                    eqm = lp.tile([128, 160], U8, tag=f"eqm{g}")
                    eqm_v = eqm[:].rearrange("p (kk b c) -> p kk b c", kk=2, b=8)
                    nc.vector.tensor_tensor(eqm_v, sc_v,
                                            smax_v.unsqueeze(3).broadcast_to([128, 2, 8, CORPUS]),
                                            op=mybir.AluOpType.is_equal)
                    cand = lp.tile([128, 160], F32, tag=f"cand{g}")
                    cand_v = cand[:].rearrange("p (kk b c) -> p kk b c", kk=2, b=8)
                    iota_b = ciota[:].unsqueeze(1).unsqueeze(1).broadcast_to([128, 2, 8, CORPUS])
                    big_b = cbig[:].unsqueeze(1).unsqueeze(1).broadcast_to([128, 2, 8, CORPUS])
                    nc.vector.select(cand_v, eqm_v, iota_b, big_b)
                    idxB = lp.tile([128, 16], F32, tag=f"idxB{g}")
                    idxB_v = idxB[:].rearrange("p (kk b) -> p kk b", kk=2)
                    nc.vector.tensor_reduce(idxB_v, cand_v, axis=mybir.AxisListType.X,
                                            op=mybir.AluOpType.min)

                    # transport idx into ap_gather wrapped layout
                    itp = ips_pool.tile([16, 128], F32, tag="itp")
                    pe_touch(itp)
                    nc.tensor.transpose(itp[:], idxB[:], cid[:])
                    idxT = lp.tile([16, 128], F32, tag=f"idxT{g}")
                    nc.scalar.activation(idxT[:], itp[:], _AFT.Copy)
                    pps2 = ips_pool.tile([128, 16], F32, tag="pps2")
                    pe_touch(pps2)
                    idxT_v = idxT[:].rearrange("p (s par) -> p par s", par=8)
                    for par in range(8):
                        nc.tensor.matmul(pps2[:], lhsT=cperm[:, 128 * par:128 * (par + 1)],
                                         rhs=idxT_v[:, par:par + 1, :],
                                         start=(par == 0), stop=(par == 7))
                    idxs16 = lp.tile([128, 16], I16, tag=f"idxs16{g}")
                    nc.vector.tensor_add(idxs16[:], pps2[:], coffs[:])

                    # gather picks into the output slot; update S stripe
                    iv = outitems[:, ki * NB + g * 256:ki * NB + (g + 1) * 256]
                    nc.gpsimd.ap_gather(iv.rearrange("p (n d) -> p n d", d=1),
                                        corpus[:, g * 2560:(g + 1) * 2560].rearrange(
                                            "p (n d) -> p n d", d=1),
                                        idxs16[:], channels=128, num_elems=2560,
                                        d=1, num_idxs=256)
                    nc.vector.tensor_tensor(S[:, 256 * g:256 * (g + 1)],
                                            S[:, 256 * g:256 * (g + 1)], iv,
                                            op=mybir.AluOpType.add)

        nc.sync.dma_start(out_d[:], outitems[:])
    # move_matmul_waits_to_ldweights drops waits from self-loading fp32
    # matmuls (no standalone LDWEIGHTS exists) -> lost deps / races.
    # generate_event_semaphores alone legalizes the 1-wait-per-inst limit.
    nc.move_matmul_waits_to_ldweights = lambda: None
    if not nc.is_finalized():
        nc.finalize()
    return nc


_NC_CACHE = {}


def kernel(user_intent, item_corpus, W_proj, b_proj, W_s, W_t, search_num,
           _trace=False):
    sn = int(search_num)
    user_intent = np.asarray(user_intent, dtype=np.float32)
    item_corpus = np.asarray(item_corpus, dtype=np.float32)
    consts = build_consts(np.asarray(W_proj, dtype=np.float32),
                          np.asarray(b_proj, dtype=np.float32),
                          np.asarray(W_s, dtype=np.float32),
                          np.asarray(W_t, dtype=np.float32), sn)

    if sn not in _NC_CACHE:
        _NC_CACHE[sn] = build_nc(sn)
    nc = _NC_CACHE[sn]

    # host prep per core
    in_maps = []
    for cc in range(NCORES):
        rows = item_corpus[cc * R:(cc + 1) * R]          # [R, 10, 100]
        arr = rows.reshape(8, NB, CORPUS, IND)            # [b_lo, b_hi, c, d]
        itT = np.empty((8, 101, NELEM), dtype=np.float32)
        itT[:, :100, :] = arr.transpose(0, 3, 1, 2).reshape(8, IND, NELEM)
        itT[:, 100, :] = 1.0
        u = user_intent[cc * R:(cc + 1) * R]              # [R, 16]
        uT = u.reshape(8, NB, HID).transpose(0, 2, 1).reshape(128, NB)
        m = dict(itemT=np.ascontiguousarray(itT), uT=np.ascontiguousarray(uT))
        m.update({k: v for k, v in consts.items()})
        in_maps.append(m)

    res = run_bass_kernel_spmd(nc, in_maps, list(range(NCORES)), trace=_trace)
    if _trace:
        kernel._last_results = res

    # host post: out [128, sn*NB] layout C -> picks [R, sn, 16]
    out = np.empty((BS, 1 + sn, HID), dtype=np.float32)
    out[:, 0, :] = user_intent
    for cc in range(NCORES):
        o = res.results[cc]["out"]                        # [128, sn*NB]
        picks = o.reshape(8, HID, sn, NB).transpose(0, 3, 2, 1).reshape(R, sn, HID)
        out[cc * R:(cc + 1) * R, 1:, :] = picks
    return out


# revision 24
# speedup vs baseline: 1.0845x; 1.0845x over previous
"""AttnGreedySearchV2 Trainium2 kernel.

Math (per batch row):
  corpus = item @ W_proj + b_proj          [10, 16]
  t_vec  = tanh(corpus @ W_t)              [10, 16]
  S = u; for k in 0..sn-1:
      s = tanh((S / (k+1)) @ W_s)          [16]
      c* = argmax_c <t_vec[c], s>
      pick v_k = corpus[c*]; S += v_k
  out = [u, v_0..v_{sn-1}]                 [1+sn, 16]

Device layout (per core, R=8192 rows, r = b_lo*1024 + b_hi):
  layout C:  partition p = 16*b_lo + h, free col = b_hi           (corpus, S, s)
  layout B': partition q, free (k, ...) with b_hi = 8*q + k       (t_vec, scores)
Scores/argmax run in B' (h on free dim -> DVE segmented reduce); the argmax
index is moved into ap_gather's per-core wrapped format with one PE transpose
plus two constant permutation matmuls.
"""

import numpy as np
from contextlib import ExitStack

import concourse.bass as bass
import concourse.bacc as bacc
import concourse.tile as tile
from concourse import mybir
from concourse.bass_utils import run_bass_kernel_spmd

F32 = mybir.dt.float32
I16 = mybir.dt.int16
U8 = mybir.dt.uint8

NCORES = 8
BS = 65536
R = BS // NCORES          # 8192 rows per core
NB = R // 8               # 1024 (b_hi)
CORPUS = 10
HID = 16
IND = 100
NELEM = NB * CORPUS       # 10240 free cols in layout C (b_hi, c)

_AFT = mybir.ActivationFunctionType


def _blockdiag(w):
    out = np.zeros((128, 128), dtype=np.float32)
    for b in range(8):
        out[16 * b:16 * b + 16, 16 * b:16 * b + 16] = w
    return out


def build_consts(W_proj, b_proj, W_s, W_t, sn):
    # wproj: [101, 8*128]; block b at cols [128b:128b+128) holds W_eff in
    # cols [16b:16b+16) of that block; row 100 is the bias row.
    wproj = np.zeros((101, 8 * 128), dtype=np.float32)
    for b in range(8):
        wproj[0:100, 128 * b + 16 * b:128 * b + 16 * b + 16] = W_proj
        wproj[100, 128 * b + 16 * b:128 * b + 16 * b + 16] = b_proj
    wt = _blockdiag(W_t)
    ws = np.zeros((128, sn * 128), dtype=np.float32)
    for k in range(sn):
        ws[:, 128 * k:128 * (k + 1)] = _blockdiag(W_s / float(k + 1))
    ident = np.eye(128, dtype=np.float32)
    # per-stripe perm: P2[par][f=8*kl+b_lo, 16*b_lo + 2*par + kl] = 1, par=q%8
    perm = np.zeros((16, 8 * 128), dtype=np.float32)
    for kl in range(2):
        for b_lo in range(8):
            for par in range(8):
                perm[8 * kl + b_lo, 128 * par + 16 * b_lo + 2 * par + kl] = 1.0
    # offs[p, s] = 10*(16*s + p%16): local elem index base within a stripe
    p = np.arange(128)[:, None]
    s = np.arange(16)[None, :]
    offs = (10.0 * (16 * s + (p % 16))).astype(np.float32)   # [128, 16]
    iotac = np.tile(np.arange(CORPUS, dtype=np.float32), (128, 1))
    big = np.full((128, 1), 1e9, dtype=np.float32)
    return dict(wproj=wproj, wt=wt, ws=ws, ident=ident, perm=perm,
                offs=offs, iotac=iotac, big=big)


def build_nc(sn, debug=False):
    nc = bacc.Bacc(None, target_bir_lowering=False)
    itemT = nc.declare_dram_parameter("itemT", [8, 101, NELEM], F32, isOutput=False)
    uT = nc.declare_dram_parameter("uT", [128, NB], F32, isOutput=False)
    wproj_d = nc.declare_dram_parameter("wproj", [101, 8 * 128], F32, isOutput=False)
    wt_d = nc.declare_dram_parameter("wt", [128, 128], F32, isOutput=False)
    ws_d = nc.declare_dram_parameter("ws", [128, sn * 128], F32, isOutput=False)
    ident_d = nc.declare_dram_parameter("ident", [128, 128], F32, isOutput=False)
    perm_d = nc.declare_dram_parameter("perm", [16, 8 * 128], F32, isOutput=False)
    offs_d = nc.declare_dram_parameter("offs", [128, 16], F32, isOutput=False)
    iotac_d = nc.declare_dram_parameter("iotac", [128, CORPUS], F32, isOutput=False)
    big_d = nc.declare_dram_parameter("big", [128, 1], F32, isOutput=False)
    out_d = nc.declare_dram_parameter("out", [128, sn * NB], F32, isOutput=True)
    if debug:
        dbg_corpus = nc.declare_dram_parameter("dbg_corpus", [128, NELEM], F32, isOutput=True)
        dbg_tB = nc.declare_dram_parameter("dbg_tB", [128, NELEM], F32, isOutput=True)
        dbg_sC = nc.declare_dram_parameter("dbg_sC", [128, NB], F32, isOutput=True)
        dbg_sB = nc.declare_dram_parameter("dbg_sB", [128, NB], F32, isOutput=True)
        dbg_scores = nc.declare_dram_parameter("dbg_scores", [128, 640], F32, isOutput=True)
        dbg_idxB = nc.declare_dram_parameter("dbg_idxB", [128, 64], F32, isOutput=True)
        dbg_idxs = nc.declare_dram_parameter("dbg_idxs", [128, NB // 16], I16, isOutput=True)

    with tile.TileContext(nc) as tc, ExitStack() as ctx:
        cpool = ctx.enter_context(tc.tile_pool(name="consts", bufs=1))
        cwproj = cpool.tile([101, 8 * 128], F32, tag="cwproj")
        nc.sync.dma_start(cwproj[:], wproj_d[:])
        cwt = cpool.tile([128, 128], F32, tag="cwt")
        nc.sync.dma_start(cwt[:], wt_d[:])
        cws = cpool.tile([128, sn * 128], F32, tag="cws")
        nc.sync.dma_start(cws[:], ws_d[:])
        cid = cpool.tile([128, 128], F32, tag="cid")
        nc.sync.dma_start(cid[:], ident_d[:])
        cperm = cpool.tile([16, 8 * 128], F32, tag="cperm")
        nc.sync.dma_start(cperm[:], perm_d[:])
        coffs = cpool.tile([128, 16], F32, tag="coffs")
        nc.sync.dma_start(coffs[:], offs_d[:])
        ciota = cpool.tile([128, CORPUS], F32, tag="ciota")
        nc.sync.dma_start(ciota[:], iotac_d[:])
        cbig = cpool.tile([128, 1], F32, tag="cbig")
        nc.sync.dma_start(cbig[:], big_d[:])

        # PE warm-up touches: make the PE observe each const's DMA lane via
        # tiny 1x1 matmuls so real matmuls carry at most one sync wait
        # (walrus S3_LW struct limit for fp32 matmuls).
        def pe_touch(tgt, src_ap=None):
            a = (src_ap if src_ap is not None else cid)[0:1, 0:1]
            nc.tensor.matmul(tgt[0:1, 0:1], lhsT=a, rhs=a, start=True, stop=True)
        with tc.tile_pool(name="warmps", bufs=1, space="PSUM") as wps_pool:
            wtile = wps_pool.tile([1, 1], F32, tag="wt")
            for cst in (cwproj, cwt, cid, cws, cperm):
                pe_touch(wtile, cst)

        main = ctx.enter_context(tc.tile_pool(name="main", bufs=1))
        corpus = main.tile([128, NELEM], F32, tag="corpus")
        tB = main.tile([128, NELEM], F32, tag="tB")
        outitems = main.tile([128, sn * NB], F32, tag="outitems")
        S = main.tile([128, NB], F32, tag="S")
        nc.sync.dma_start(S[:], uT[:])

        # iteration-0 s-stage depends only on S=u: emit before the projection
        sB0 = {}
        with tc.tile_pool(name="hpsum", bufs=1, space="PSUM") as hps:
            for g in range(4):
                hsps = hps.tile([128, 256], F32, tag="hsps")
                pe_touch(hsps)
                nc.tensor.matmul(hsps[:], lhsT=cws[:, 0:128],
                                 rhs=S[:, 256 * g:256 * (g + 1)],
                                 start=True, stop=True)
                hsC = main.tile([128, 256], F32, tag=f"hsC{g}")
                nc.scalar.activation(hsC[:], hsps[:], _AFT.Tanh)
                hsbp = hps.tile([128, 256], F32, tag="hsbp")
                hsC_v = hsC[:].rearrange("p (q kk) -> p kk q", kk=2)
                for kl in range(2):
                    nc.tensor.transpose(hsbp[:, 128 * kl:128 * (kl + 1)],
                                        hsC_v[:, kl, :], cid[:])
                sB0g = main.tile([128, 256], F32, tag=f"hsB{g}")
                nc.scalar.activation(sB0g[:], hsbp[:], _AFT.Copy)
                sB0[g] = sB0g
        # ------- loop pools (open across the whole program) -------
        lp = ctx.enter_context(tc.tile_pool(name="loop", bufs=2))
        prp = ctx.enter_context(tc.tile_pool(name="prp", bufs=2))
        sps_pool = ctx.enter_context(tc.tile_pool(name="spsum", bufs=2, space="PSUM"))
        ips_pool = ctx.enter_context(tc.tile_pool(name="ipsum", bufs=1, space="PSUM"))

        def s_stage(ki, g):
            # s = tanh(BD(W_s/(ki+1)) @ S_g)
            sps = sps_pool.tile([128, 256], F32, tag="sps")
            pe_touch(sps)
            nc.tensor.matmul(sps[:], lhsT=cws[:, 128 * ki:128 * (ki + 1)],
                             rhs=S[:, 256 * g:256 * (g + 1)],
                             start=True, stop=True)
            sC = lp.tile([128, 256], F32, tag=f"sC{g}")
            nc.scalar.activation(sC[:], sps[:], _AFT.Tanh)
            sbp = sps_pool.tile([128, 256], F32, tag="sps")
            sC_v = sC[:].rearrange("p (q kk) -> p kk q", kk=2)
            for kl in range(2):
                nc.tensor.transpose(sbp[:, 128 * kl:128 * (kl + 1)],
                                    sC_v[:, kl, :], cid[:])
            sB = lp.tile([128, 256], F32, tag=f"sB{g}")
            nc.scalar.activation(sB[:], sbp[:], _AFT.Copy)
            return sB

        def body(ki, g, sB):
            # scores for k in {2g, 2g+1}, one chunk per k-window
            scores = lp.tile([128, 160], F32, tag=f"scores{g}")
            for kl in range(2):
                pr = prp.tile([128, 1280], F32, tag=f"pr{g % 2}")
                t_in = tB[:, (2 * g + kl) * 1280:(2 * g + kl + 1) * 1280].rearrange(
                    "p (c bh) -> p c bh", c=CORPUS)
                s_in = sB[:, 128 * kl:128 * (kl + 1)].unsqueeze(1).broadcast_to(
                    [128, CORPUS, 128])
                pr_v = pr[:].rearrange("p (c bh) -> p c bh", c=CORPUS)
                nc.vector.tensor_mul(pr_v, t_in, s_in)
                red_in = pr[:].rearrange("p (a h) -> p a h", h=HID)
                nc.vector.reduce_sum(scores[:, 80 * kl:80 * (kl + 1)],
                                     red_in, axis=mybir.AxisListType.X)
            # argmax over c (first max)
            sc_v = scores[:].rearrange("p (kk c b) -> p kk c b",
                                       kk=2, c=CORPUS).transpose([0, 1, 3, 2])
            smax = lp.tile([128, 16], F32, tag=f"smax{g}")
            smax_v = smax[:].rearrange("p (kk b) -> p kk b", kk=2)
            nc.vector.reduce_max(smax_v, sc_v, axis=mybir.AxisListType.X)
            eqm = lp.tile([128, 160], U8, tag=f"eqm{g}")
            eqm_v = eqm[:].rearrange("p (kk b c) -> p kk b c", kk=2, b=8)
            nc.vector.tensor_tensor(eqm_v, sc_v,
                                    smax_v.unsqueeze(3).broadcast_to([128, 2, 8, CORPUS]),
                                    op=mybir.AluOpType.is_equal)
            cand = lp.tile([128, 160], F32, tag=f"cand{g}")
            cand_v = cand[:].rearrange("p (kk b c) -> p kk b c", kk=2, b=8)
            iota_b = ciota[:].unsqueeze(1).unsqueeze(1).broadcast_to([128, 2, 8, CORPUS])
            big_b = cbig[:].unsqueeze(1).unsqueeze(1).broadcast_to([128, 2, 8, CORPUS])
            nc.vector.select(cand_v, eqm_v, iota_b, big_b)
            idxB = lp.tile([128, 16], F32, tag=f"idxB{g}")
            idxB_v = idxB[:].rearrange("p (kk b) -> p kk b", kk=2)
            nc.vector.tensor_reduce(idxB_v, cand_v, axis=mybir.AxisListType.X,
                                    op=mybir.AluOpType.min)
            # transport idx into ap_gather wrapped layout (itp and pps2 share
            # one PSUM slot; the WAR dep serializes them naturally)
            itp = ips_pool.tile([16, 128], F32, tag="ipsu")
            pe_touch(itp)
            nc.tensor.transpose(itp[:], idxB[:], cid[:])
            idxT = lp.tile([16, 128], F32, tag=f"idxT{g}")
            nc.scalar.activation(idxT[:], itp[:], _AFT.Copy)
            pps2 = ips_pool.tile([128, 16], F32, tag="ipsu")
            pe_touch(pps2)
            idxT_v = idxT[:].rearrange("p (s par) -> p par s", par=8)
            for par in range(8):
                nc.tensor.matmul(pps2[:], lhsT=cperm[:, 128 * par:128 * (par + 1)],
                                 rhs=idxT_v[:, par:par + 1, :],
                                 start=(par == 0), stop=(par == 7))
            idxs16 = lp.tile([128, 16], I16, tag=f"idxs16{g}")
            nc.vector.tensor_add(idxs16[:], pps2[:], coffs[:])
            # gather picks into the output slot; S update on GPSIMD (same
            # engine as the gather -> no cross-engine hop)
            iv = outitems[:, ki * NB + g * 256:ki * NB + (g + 1) * 256]
            nc.gpsimd.ap_gather(iv.rearrange("p (n d) -> p n d", d=1),
                                corpus[:, g * 2560:(g + 1) * 2560].rearrange(
                                    "p (n d) -> p n d", d=1),
                                idxs16[:], channels=128, num_elems=2560,
                                d=1, num_idxs=256)
            nc.gpsimd.tensor_tensor(S[:, 256 * g:256 * (g + 1)],
                                    S[:, 256 * g:256 * (g + 1)], iv,
                                    op=mybir.AluOpType.add)

        sB_cur = dict(sB0)

        def wave_tail(w):
            # emit loop stages whose inputs became ready during wave w-1
            for g2 in range(4):
                ki = w - 1 - g2
                if 0 <= ki < sn:
                    body(ki, g2, sB_cur[g2])
                    if ki + 1 < sn:
                        sB_cur[g2] = s_stage(ki + 1, g2)

        # ------- projection + t-phase per stripe, loop waves interleaved ----
        W = 256
        itemT_v = itemT[:].rearrange("b d n -> d b n")
        qchunks = [(0, 51), (51, 51), (102, 26)]
        with tc.tile_pool(name="proj", bufs=2) as ppool, \
             tc.tile_pool(name="ppsum", bufs=1, space="PSUM") as pps, \
             tc.tile_pool(name="tch", bufs=2) as tchp, \
             tc.tile_pool(name="tpsum", bufs=1, space="PSUM") as tps_pool, \
             tc.tile_pool(name="tbpsum", bufs=1, space="PSUM") as tbp_pool:
            for g in range(4):
                for wl in range(10):
                    w = g * 10 + wl
                    it = ppool.tile([101, 8 * W], F32, tag="it")
                    itv = it[:].rearrange("d (b n) -> d b n", b=8)
                    nc.sync.dma_start(itv, itemT_v[:, :, w * W:(w + 1) * W])
                    ps = pps.tile([128, W], F32, tag="ps")
                    pe_touch(ps)
                    for b in range(8):
                        nc.tensor.matmul(ps[:], lhsT=cwproj[:, 128 * b:128 * b + 128],
                                         rhs=it[:, b * W:(b + 1) * W],
                                         start=(b == 0), stop=(b == 7))
                    nc.scalar.activation(corpus[:, w * W:(w + 1) * W], ps[:], _AFT.Copy)
                for klw in range(2):
                    k = 2 * g + klw
                    blk = corpus[:, g * 2560:(g + 1) * 2560]
                    corpus_vk = blk.rearrange("p (q kk c) -> p kk q c", kk=2, c=CORPUS)
                    # two c-halves so PSUM tiles stay small (2 banks each)
                    for ch in range(2):
                        c0 = 5 * ch
                        tps = tps_pool.tile([128, 1024], F32, tag="tps")
                        pe_touch(tps)
                        # q-chunks of 102/26 -> 510/130 cols, bank-aligned
                        nc.tensor.matmul(tps[:, 0:510], lhsT=cwt[:],
                                         rhs=corpus_vk[:, klw, 0:102, c0:c0 + 5],
                                         start=True, stop=True)
                        nc.tensor.matmul(tps[:, 512:642], lhsT=cwt[:],
                                         rhs=corpus_vk[:, klw, 102:128, c0:c0 + 5],
                                         start=True, stop=True)
                        tch = tchp.tile([128, 640], F32, tag="tch")
                        nc.scalar.activation(tch[:, 0:510], tps[:, 0:510], _AFT.Tanh)
                        nc.scalar.activation(tch[:, 510:640], tps[:, 512:642], _AFT.Tanh)
                        tbp = tbp_pool.tile([128, 640], F32, tag="tbp")
                        tch_v = tch[:].rearrange("p (q c) -> p c q", c=5)
                        for c in range(5):
                            nc.tensor.transpose(tbp[:, 128 * c:128 * (c + 1)],
                                                tch_v[:, c, :], cid[:])
                        nc.scalar.activation(
                            tB[:, 1280 * k + 640 * ch:1280 * k + 640 * (ch + 1)],
                            tbp[:], _AFT.Copy)
                wave_tail(g + 1)

        for w in range(5, 4 + sn + 1):
            wave_tail(w)

        nc.sync.dma_start(out_d[:], outitems[:])
    # move_matmul_waits_to_ldweights drops waits from self-loading fp32
    # matmuls (no standalone LDWEIGHTS exists) -> lost deps / races.
    # generate_event_semaphores alone legalizes the 1-wait-per-inst limit.
    nc.move_matmul_waits_to_ldweights = lambda: None
    if not nc.is_finalized():
        nc.finalize()
    return nc


_NC_CACHE = {}


def kernel(user_intent, item_corpus, W_proj, b_proj, W_s, W_t, search_num,
           _trace=False):
    sn = int(search_num)
    user_intent = np.asarray(user_intent, dtype=np.float32)
    item_corpus = np.asarray(item_corpus, dtype=np.float32)
    consts = build_consts(np.asarray(W_proj, dtype=np.float32),
                          np.asarray(b_proj, dtype=np.float32),
                          np.asarray(W_s, dtype=np.float32),
                          np.asarray(W_t, dtype=np.float32), sn)

    if sn not in _NC_CACHE:
        _NC_CACHE[sn] = build_nc(sn)
    nc = _NC_CACHE[sn]

    # host prep per core
    in_maps = []
    for cc in range(NCORES):
        rows = item_corpus[cc * R:(cc + 1) * R]          # [R, 10, 100]
        arr = rows.reshape(8, NB, CORPUS, IND)            # [b_lo, b_hi, c, d]
        itT = np.empty((8, 101, NELEM), dtype=np.float32)
        itT[:, :100, :] = arr.transpose(0, 3, 1, 2).reshape(8, IND, NELEM)
        itT[:, 100, :] = 1.0
        u = user_intent[cc * R:(cc + 1) * R]              # [R, 16]
        uT = u.reshape(8, NB, HID).transpose(0, 2, 1).reshape(128, NB)
        m = dict(itemT=np.ascontiguousarray(itT), uT=np.ascontiguousarray(uT))
        m.update({k: v for k, v in consts.items()})
        in_maps.append(m)

    res = run_bass_kernel_spmd(nc, in_maps, list(range(NCORES)), trace=_trace)
    if _trace:
        kernel._last_results = res

    # host post: out [128, sn*NB] layout C -> picks [R, sn, 16]
    out = np.empty((BS, 1 + sn, HID), dtype=np.float32)
    out[:, 0, :] = user_intent
    for cc in range(NCORES):
        o = res.results[cc]["out"]                        # [128, sn*NB]
        picks = o.reshape(8, HID, sn, NB).transpose(0, 3, 2, 1).reshape(R, sn, HID)
        out[cc * R:(cc + 1) * R, 1:, :] = picks
    return out


# revision 27
# speedup vs baseline: 1.1041x; 1.0181x over previous
"""AttnGreedySearchV2 Trainium2 kernel.

Math (per batch row):
  corpus = item @ W_proj + b_proj          [10, 16]
  t_vec  = tanh(corpus @ W_t)              [10, 16]
  S = u; for k in 0..sn-1:
      s = tanh((S / (k+1)) @ W_s)          [16]
      c* = argmax_c <t_vec[c], s>
      pick v_k = corpus[c*]; S += v_k
  out = [u, v_0..v_{sn-1}]                 [1+sn, 16]

Device layout (per core, R=8192 rows, r = b_lo*1024 + b_hi):
  layout C:  partition p = 16*b_lo + h, free col = b_hi           (corpus, S, s)
  layout B': partition q, free (k, ...) with b_hi = 8*q + k       (t_vec, scores)
Scores/argmax run in B' (h on free dim -> DVE segmented reduce); the argmax
index is moved into ap_gather's per-core wrapped format with one PE transpose
plus two constant permutation matmuls.
"""

import numpy as np
from contextlib import ExitStack

import concourse.bass as bass
import concourse.bacc as bacc
import concourse.tile as tile
from concourse import mybir
from concourse.bass_utils import run_bass_kernel_spmd

F32 = mybir.dt.float32
I16 = mybir.dt.int16
U8 = mybir.dt.uint8

NCORES = 8
BS = 65536
R = BS // NCORES          # 8192 rows per core
NB = R // 8               # 1024 (b_hi)
CORPUS = 10
HID = 16
IND = 100
NELEM = NB * CORPUS       # 10240 free cols in layout C (b_hi, c)

_AFT = mybir.ActivationFunctionType


def _blockdiag(w):
    out = np.zeros((128, 128), dtype=np.float32)
    for b in range(8):
        out[16 * b:16 * b + 16, 16 * b:16 * b + 16] = w
    return out


def build_consts(W_proj, b_proj, W_s, W_t, sn):
    # wproj: [101, 8*128]; block b at cols [128b:128b+128) holds W_eff in
    # cols [16b:16b+16) of that block; row 100 is the bias row.
    wproj = np.zeros((101, 8 * 128), dtype=np.float32)
    for b in range(8):
        wproj[0:100, 128 * b + 16 * b:128 * b + 16 * b + 16] = W_proj
        wproj[100, 128 * b + 16 * b:128 * b + 16 * b + 16] = b_proj
    wt = _blockdiag(W_t)
    ws = np.zeros((128, sn * 128), dtype=np.float32)
    for k in range(sn):
        ws[:, 128 * k:128 * (k + 1)] = _blockdiag(W_s / float(k + 1))
    ident = np.eye(128, dtype=np.float32)
    # per-stripe perm: P2[par][f=8*kl+b_lo, 16*b_lo + 2*par + kl] = 1, par=q%8
    perm = np.zeros((16, 8 * 128), dtype=np.float32)
    for kl in range(2):
        for b_lo in range(8):
            for par in range(8):
                perm[8 * kl + b_lo, 128 * par + 16 * b_lo + 2 * par + kl] = 1.0
    # offs[p, s] = 10*(16*s + p%16): local elem index base within a stripe
    p = np.arange(128)[:, None]
    s = np.arange(16)[None, :]
    offs = (10.0 * (16 * s + (p % 16))).astype(np.float32)   # [128, 16]
    iotac = np.tile(np.arange(CORPUS, dtype=np.float32), (128, 1))
    big = np.full((128, 1), 1e9, dtype=np.float32)
    return dict(wproj=wproj, wt=wt, ws=ws, ident=ident, perm=perm,
                offs=offs, iotac=iotac, big=big)


def build_nc(sn, debug=False):
    nc = bacc.Bacc(None, target_bir_lowering=False)
    itemT = nc.declare_dram_parameter("itemT", [8, 101, NELEM], F32, isOutput=False)
    uT = nc.declare_dram_parameter("uT", [128, NB], F32, isOutput=False)
    wproj_d = nc.declare_dram_parameter("wproj", [101, 8 * 128], F32, isOutput=False)
    wt_d = nc.declare_dram_parameter("wt", [128, 128], F32, isOutput=False)
    ws_d = nc.declare_dram_parameter("ws", [128, sn * 128], F32, isOutput=False)
    ident_d = nc.declare_dram_parameter("ident", [128, 128], F32, isOutput=False)
    perm_d = nc.declare_dram_parameter("perm", [16, 8 * 128], F32, isOutput=False)
    offs_d = nc.declare_dram_parameter("offs", [128, 16], F32, isOutput=False)
    iotac_d = nc.declare_dram_parameter("iotac", [128, CORPUS], F32, isOutput=False)
    big_d = nc.declare_dram_parameter("big", [128, 1], F32, isOutput=False)
    out_d = nc.declare_dram_parameter("out", [128, sn * NB], F32, isOutput=True)
    if debug:
        dbg_corpus = nc.declare_dram_parameter("dbg_corpus", [128, NELEM], F32, isOutput=True)
        dbg_tB = nc.declare_dram_parameter("dbg_tB", [128, NELEM], F32, isOutput=True)
        dbg_sC = nc.declare_dram_parameter("dbg_sC", [128, NB], F32, isOutput=True)
        dbg_sB = nc.declare_dram_parameter("dbg_sB", [128, NB], F32, isOutput=True)
        dbg_scores = nc.declare_dram_parameter("dbg_scores", [128, 640], F32, isOutput=True)
        dbg_idxB = nc.declare_dram_parameter("dbg_idxB", [128, 64], F32, isOutput=True)
        dbg_idxs = nc.declare_dram_parameter("dbg_idxs", [128, NB // 16], I16, isOutput=True)

    with tile.TileContext(nc) as tc, ExitStack() as ctx:
        cpool = ctx.enter_context(tc.tile_pool(name="consts", bufs=1))
        cwproj = cpool.tile([101, 8 * 128], F32, tag="cwproj")
        nc.sync.dma_start(cwproj[:], wproj_d[:])
        cwt = cpool.tile([128, 128], F32, tag="cwt")
        nc.sync.dma_start(cwt[:], wt_d[:])
        cws = cpool.tile([128, sn * 128], F32, tag="cws")
        nc.sync.dma_start(cws[:], ws_d[:])
        cid = cpool.tile([128, 128], F32, tag="cid")
        nc.sync.dma_start(cid[:], ident_d[:])
        cperm = cpool.tile([16, 8 * 128], F32, tag="cperm")
        nc.sync.dma_start(cperm[:], perm_d[:])
        coffs = cpool.tile([128, 16], F32, tag="coffs")
        nc.sync.dma_start(coffs[:], offs_d[:])
        ciota = cpool.tile([128, CORPUS], F32, tag="ciota")
        nc.sync.dma_start(ciota[:], iotac_d[:])
        cbig = cpool.tile([128, 1], F32, tag="cbig")
        nc.sync.dma_start(cbig[:], big_d[:])

        # PE warm-up touches: make the PE observe each const's DMA lane via
        # tiny 1x1 matmuls so real matmuls carry at most one sync wait
        # (walrus S3_LW struct limit for fp32 matmuls).
        def pe_touch(tgt, src_ap=None):
            a = (src_ap if src_ap is not None else cid)[0:1, 0:1]
            nc.tensor.matmul(tgt[0:1, 0:1], lhsT=a, rhs=a, start=True, stop=True)
        with tc.tile_pool(name="warmps", bufs=1, space="PSUM") as wps_pool:
            wtile = wps_pool.tile([1, 1], F32, tag="wt")
            for cst in (cwproj, cwt, cid, cws, cperm):
                pe_touch(wtile, cst)

        main = ctx.enter_context(tc.tile_pool(name="main", bufs=1))
        corpus = main.tile([128, NELEM], F32, tag="corpus")
        tB = main.tile([128, NELEM], F32, tag="tB")
        outitems = main.tile([128, sn * NB], F32, tag="outitems")
        S = main.tile([128, NB], F32, tag="S")
        nc.sync.dma_start(S[:], uT[:])

        # iteration-0 s-stage depends only on S=u: emit before the projection
        sB0 = {}
        with tc.tile_pool(name="hpsum", bufs=1, space="PSUM") as hps:
            for g in range(4):
                hsps = hps.tile([128, 256], F32, tag="hsps")
                pe_touch(hsps)
                nc.tensor.matmul(hsps[:], lhsT=cws[:, 0:128],
                                 rhs=S[:, 256 * g:256 * (g + 1)],
                                 start=True, stop=True)
                hsC = main.tile([128, 256], F32, tag=f"hsC{g}")
                nc.scalar.activation(hsC[:], hsps[:], _AFT.Tanh)
                hsbp = hps.tile([128, 256], F32, tag="hsbp")
                hsC_v = hsC[:].rearrange("p (q kk) -> p kk q", kk=2)
                for kl in range(2):
                    nc.tensor.transpose(hsbp[:, 128 * kl:128 * (kl + 1)],
                                        hsC_v[:, kl, :], cid[:])
                sB0g = main.tile([128, 256], F32, tag=f"hsB{g}")
                nc.scalar.activation(sB0g[:], hsbp[:], _AFT.Copy)
                sB0[g] = sB0g
        # ------- loop pools (open across the whole program) -------
        lp = ctx.enter_context(tc.tile_pool(name="loop", bufs=2))
        prp = ctx.enter_context(tc.tile_pool(name="prp", bufs=2))
        sps_pool = ctx.enter_context(tc.tile_pool(name="spsum", bufs=2, space="PSUM"))
        ips_pool = ctx.enter_context(tc.tile_pool(name="ipsum", bufs=1, space="PSUM"))

        def s_stage(ki, g):
            # s = tanh(BD(W_s/(ki+1)) @ S_g)
            sps = sps_pool.tile([128, 256], F32, tag="sps")
            pe_touch(sps)
            nc.tensor.matmul(sps[:], lhsT=cws[:, 128 * ki:128 * (ki + 1)],
                             rhs=S[:, 256 * g:256 * (g + 1)],
                             start=True, stop=True)
            sC = lp.tile([128, 256], F32, tag=f"sC{g}")
            nc.scalar.activation(sC[:], sps[:], _AFT.Tanh)
            sbp = sps_pool.tile([128, 256], F32, tag="sps")
            sC_v = sC[:].rearrange("p (q kk) -> p kk q", kk=2)
            for kl in range(2):
                nc.tensor.transpose(sbp[:, 128 * kl:128 * (kl + 1)],
                                    sC_v[:, kl, :], cid[:])
            sB = lp.tile([128, 256], F32, tag=f"sB{g}")
            nc.scalar.activation(sB[:], sbp[:], _AFT.Copy)
            return sB

        def body(ki, g, sB):
            # scores for k in {2g, 2g+1}, one chunk per k-window
            scores = lp.tile([128, 160], F32, tag=f"scores{g}")
            for kl in range(2):
                pr = prp.tile([128, 1280], F32, tag=f"pr{g % 2}")
                t_in = tB[:, (2 * g + kl) * 1280:(2 * g + kl + 1) * 1280].rearrange(
                    "p (c bh) -> p c bh", c=CORPUS)
                s_in = sB[:, 128 * kl:128 * (kl + 1)].unsqueeze(1).broadcast_to(
                    [128, CORPUS, 128])
                pr_v = pr[:].rearrange("p (c bh) -> p c bh", c=CORPUS)
                nc.vector.tensor_mul(pr_v, t_in, s_in)
                red_in = pr[:].rearrange("p (a h) -> p a h", h=HID)
                nc.vector.reduce_sum(scores[:, 80 * kl:80 * (kl + 1)],
                                     red_in, axis=mybir.AxisListType.X)
            # argmax over c (first max)
            sc_v = scores[:].rearrange("p (kk c b) -> p kk c b",
                                       kk=2, c=CORPUS).transpose([0, 1, 3, 2])
            smax = lp.tile([128, 16], F32, tag=f"smax{g}")
            smax_v = smax[:].rearrange("p (kk b) -> p kk b", kk=2)
            nc.vector.reduce_max(smax_v, sc_v, axis=mybir.AxisListType.X)
            eqm = lp.tile([128, 160], U8, tag=f"eqm{g}")
            eqm_v = eqm[:].rearrange("p (kk b c) -> p kk b c", kk=2, b=8)
            nc.vector.tensor_tensor(eqm_v, sc_v,
                                    smax_v.unsqueeze(3).broadcast_to([128, 2, 8, CORPUS]),
                                    op=mybir.AluOpType.is_equal)
            cand = lp.tile([128, 160], F32, tag=f"cand{g}")
            cand_v = cand[:].rearrange("p (kk b c) -> p kk b c", kk=2, b=8)
            iota_b = ciota[:].unsqueeze(1).unsqueeze(1).broadcast_to([128, 2, 8, CORPUS])
            big_b = cbig[:].unsqueeze(1).unsqueeze(1).broadcast_to([128, 2, 8, CORPUS])
            nc.vector.select(cand_v, eqm_v, iota_b, big_b)
            idxB = lp.tile([128, 16], F32, tag=f"idxB{g}")
            idxB_v = idxB[:].rearrange("p (kk b) -> p kk b", kk=2)
            nc.vector.tensor_reduce(idxB_v, cand_v, axis=mybir.AxisListType.X,
                                    op=mybir.AluOpType.min)
            # transport idx into ap_gather wrapped layout (itp and pps2 share
            # one PSUM slot; the WAR dep serializes them naturally)
            itp = ips_pool.tile([16, 128], F32, tag="ipsu")
            pe_touch(itp)
            nc.tensor.transpose(itp[:], idxB[:], cid[:])
            idxT = lp.tile([16, 128], F32, tag=f"idxT{g}")
            nc.scalar.activation(idxT[:], itp[:], _AFT.Copy)
            pps2 = ips_pool.tile([128, 16], F32, tag="ipsu")
            pe_touch(pps2)
            idxT_v = idxT[:].rearrange("p (s par) -> p par s", par=8)
            for par in range(8):
                nc.tensor.matmul(pps2[:], lhsT=cperm[:, 128 * par:128 * (par + 1)],
                                 rhs=idxT_v[:, par:par + 1, :],
                                 start=(par == 0), stop=(par == 7))
            idxs16 = lp.tile([128, 16], I16, tag=f"idxs16{g}")
            nc.vector.tensor_add(idxs16[:], pps2[:], coffs[:])
            # gather picks into the output slot; S update on GPSIMD (same
            # engine as the gather -> no cross-engine hop)
            iv = outitems[:, ki * NB + g * 256:ki * NB + (g + 1) * 256]
            nc.gpsimd.ap_gather(iv.rearrange("p (n d) -> p n d", d=1),
                                corpus[:, g * 2560:(g + 1) * 2560].rearrange(
                                    "p (n d) -> p n d", d=1),
                                idxs16[:], channels=128, num_elems=2560,
                                d=1, num_idxs=256)
            nc.gpsimd.tensor_tensor(S[:, 256 * g:256 * (g + 1)],
                                    S[:, 256 * g:256 * (g + 1)], iv,
                                    op=mybir.AluOpType.add)

        sB_cur = dict(sB0)

        def wave_tail(w):
            # emit loop stages whose inputs became ready during wave w-1
            for g2 in range(4):
                ki = w - 1 - g2
                if 0 <= ki < sn:
                    body(ki, g2, sB_cur[g2])
                    if ki + 1 < sn:
                        sB_cur[g2] = s_stage(ki + 1, g2)
                    if g2 == 3:
                        # iteration ki fully gathered: stream its output slot
                        nc.sync.dma_start(out_d[:, ki * NB:(ki + 1) * NB],
                                          outitems[:, ki * NB:(ki + 1) * NB])

        # ------- projection + t-phase per stripe, loop waves interleaved ----
        W = 256
        itemT_v = itemT[:].rearrange("b d n -> d b n")
        qchunks = [(0, 51), (51, 51), (102, 26)]
        with tc.tile_pool(name="proj", bufs=2) as ppool, \
             tc.tile_pool(name="ppsum", bufs=1, space="PSUM") as pps, \
             tc.tile_pool(name="tch", bufs=2) as tchp, \
             tc.tile_pool(name="tpsum", bufs=1, space="PSUM") as tps_pool, \
             tc.tile_pool(name="tbpsum", bufs=1, space="PSUM") as tbp_pool:
            for g in range(4):
                for wl in range(10):
                    w = g * 10 + wl
                    it = ppool.tile([101, 8 * W], F32, tag="it")
                    itv = it[:].rearrange("d (b n) -> d b n", b=8)
                    nc.sync.dma_start(itv, itemT_v[:, :, w * W:(w + 1) * W])
                    ps = pps.tile([128, W], F32, tag="ps")
                    pe_touch(ps)
                    for b in range(8):
                        nc.tensor.matmul(ps[:], lhsT=cwproj[:, 128 * b:128 * b + 128],
                                         rhs=it[:, b * W:(b + 1) * W],
                                         start=(b == 0), stop=(b == 7))
                    nc.scalar.activation(corpus[:, w * W:(w + 1) * W], ps[:], _AFT.Copy)
                for klw in range(2):
                    k = 2 * g + klw
                    blk = corpus[:, g * 2560:(g + 1) * 2560]
                    corpus_vk = blk.rearrange("p (q kk c) -> p kk q c", kk=2, c=CORPUS)
                    # two c-halves so PSUM tiles stay small (2 banks each)
                    for ch in range(2):
                        c0 = 5 * ch
                        tps = tps_pool.tile([128, 1024], F32, tag="tps")
                        pe_touch(tps)
                        # q-chunks of 102/26 -> 510/130 cols, bank-aligned
                        nc.tensor.matmul(tps[:, 0:510], lhsT=cwt[:],
                                         rhs=corpus_vk[:, klw, 0:102, c0:c0 + 5],
                                         start=True, stop=True)
                        nc.tensor.matmul(tps[:, 512:642], lhsT=cwt[:],
                                         rhs=corpus_vk[:, klw, 102:128, c0:c0 + 5],
                                         start=True, stop=True)
                        tch = tchp.tile([128, 640], F32, tag="tch")
                        nc.scalar.activation(tch[:, 0:510], tps[:, 0:510], _AFT.Tanh)
                        nc.scalar.activation(tch[:, 510:640], tps[:, 512:642], _AFT.Tanh)
                        tbp = tbp_pool.tile([128, 640], F32, tag="tbp")
                        tch_v = tch[:].rearrange("p (q c) -> p c q", c=5)
                        for c in range(5):
                            nc.tensor.transpose(tbp[:, 128 * c:128 * (c + 1)],
                                                tch_v[:, c, :], cid[:])
                        nc.scalar.activation(
                            tB[:, 1280 * k + 640 * ch:1280 * k + 640 * (ch + 1)],
                            tbp[:], _AFT.Copy)
                wave_tail(g + 1)

        for w in range(5, 4 + sn + 1):
            wave_tail(w)

    # move_matmul_waits_to_ldweights drops waits from self-loading fp32
    # matmuls (no standalone LDWEIGHTS exists) -> lost deps / races.
    # generate_event_semaphores alone legalizes the 1-wait-per-inst limit.
    nc.move_matmul_waits_to_ldweights = lambda: None
    if not nc.is_finalized():
        nc.finalize()
    return nc


_NC_CACHE = {}


def kernel(user_intent, item_corpus, W_proj, b_proj, W_s, W_t, search_num,
           _trace=False):
    sn = int(search_num)
    user_intent = np.asarray(user_intent, dtype=np.float32)
    item_corpus = np.asarray(item_corpus, dtype=np.float32)
    consts = build_consts(np.asarray(W_proj, dtype=np.float32),
                          np.asarray(b_proj, dtype=np.float32),
                          np.asarray(W_s, dtype=np.float32),
                          np.asarray(W_t, dtype=np.float32), sn)

    if sn not in _NC_CACHE:
        _NC_CACHE[sn] = build_nc(sn)
    nc = _NC_CACHE[sn]

    # host prep per core
    in_maps = []
    for cc in range(NCORES):
        rows = item_corpus[cc * R:(cc + 1) * R]          # [R, 10, 100]
        arr = rows.reshape(8, NB, CORPUS, IND)            # [b_lo, b_hi, c, d]
        itT = np.empty((8, 101, NELEM), dtype=np.float32)
        itT[:, :100, :] = arr.transpose(0, 3, 1, 2).reshape(8, IND, NELEM)
        itT[:, 100, :] = 1.0
        u = user_intent[cc * R:(cc + 1) * R]              # [R, 16]
        uT = u.reshape(8, NB, HID).transpose(0, 2, 1).reshape(128, NB)
        m = dict(itemT=np.ascontiguousarray(itT), uT=np.ascontiguousarray(uT))
        m.update({k: v for k, v in consts.items()})
        in_maps.append(m)

    res = run_bass_kernel_spmd(nc, in_maps, list(range(NCORES)), trace=_trace)
    if _trace:
        kernel._last_results = res

    # host post: out [128, sn*NB] layout C -> picks [R, sn, 16]
    out = np.empty((BS, 1 + sn, HID), dtype=np.float32)
    out[:, 0, :] = user_intent
    for cc in range(NCORES):
        o = res.results[cc]["out"]                        # [128, sn*NB]
        picks = o.reshape(8, HID, sn, NB).transpose(0, 3, 2, 1).reshape(R, sn, HID)
        out[cc * R:(cc + 1) * R, 1:, :] = picks
    return out


# revision 30
# speedup vs baseline: 1.1514x; 1.0429x over previous
"""AttnGreedySearchV2 Trainium2 kernel.

Math (per batch row):
  corpus = item @ W_proj + b_proj          [10, 16]
  t_vec  = tanh(corpus @ W_t)              [10, 16]
  S = u; for k in 0..sn-1:
      s = tanh((S / (k+1)) @ W_s)          [16]
      c* = argmax_c <t_vec[c], s>
      pick v_k = corpus[c*]; S += v_k
  out = [u, v_0..v_{sn-1}]                 [1+sn, 16]

Device layout (per core, R=8192 rows, r = b_lo*1024 + b_hi):
  layout C:  partition p = 16*b_lo + h, free col = b_hi           (corpus, S, s)
  layout B': partition q, free (k, ...) with b_hi = 8*q + k       (t_vec, scores)
Scores/argmax run in B' (h on free dim -> DVE segmented reduce); the argmax
index is moved into ap_gather's per-core wrapped format with one PE transpose
plus two constant permutation matmuls.
"""

import numpy as np
from contextlib import ExitStack

import concourse.bass as bass
import concourse.bacc as bacc
import concourse.tile as tile
from concourse import mybir
from concourse.bass_utils import run_bass_kernel_spmd

F32 = mybir.dt.float32
I16 = mybir.dt.int16
U8 = mybir.dt.uint8

NCORES = 8
BS = 65536
R = BS // NCORES          # 8192 rows per core
NB = R // 8               # 1024 (b_hi)
CORPUS = 10
HID = 16
IND = 100
NELEM = NB * CORPUS       # 10240 free cols in layout C (b_hi, c)

_AFT = mybir.ActivationFunctionType


def _blockdiag(w):
    out = np.zeros((128, 128), dtype=np.float32)
    for b in range(8):
        out[16 * b:16 * b + 16, 16 * b:16 * b + 16] = w
    return out


def build_consts(W_proj, b_proj, W_s, W_t, sn):
    # wproj: [101, 8*128]; block b at cols [128b:128b+128) holds W_eff in
    # cols [16b:16b+16) of that block; row 100 is the bias row.
    wproj = np.zeros((101, 8 * 128), dtype=np.float32)
    for b in range(8):
        wproj[0:100, 128 * b + 16 * b:128 * b + 16 * b + 16] = W_proj
        wproj[100, 128 * b + 16 * b:128 * b + 16 * b + 16] = b_proj
    wt = _blockdiag(W_t)
    ws = np.zeros((128, sn * 128), dtype=np.float32)
    for k in range(sn):
        ws[:, 128 * k:128 * (k + 1)] = _blockdiag(W_s / float(k + 1))
    ident = np.eye(128, dtype=np.float32)
    # per-stripe perm: P2[par][f=8*kl+b_lo, 16*b_lo + 2*par + kl] = 1, par=q%8
    perm = np.zeros((16, 8 * 128), dtype=np.float32)
    for kl in range(2):
        for b_lo in range(8):
            for par in range(8):
                perm[8 * kl + b_lo, 128 * par + 16 * b_lo + 2 * par + kl] = 1.0
    # offs[p, s] = 10*(16*s + p%16): local elem index base within a stripe
    p = np.arange(128)[:, None]
    s = np.arange(16)[None, :]
    offs = (10.0 * (16 * s + (p % 16))).astype(np.float32)   # [128, 16]
    iotac = np.tile(np.arange(CORPUS, dtype=np.float32), (128, 1))
    big = np.full((128, 1), 1e9, dtype=np.float32)
    return dict(wproj=wproj, wt=wt, ws=ws, ident=ident, perm=perm,
                offs=offs, iotac=iotac, big=big)


def build_nc(sn, debug=False):
    nc = bacc.Bacc(None, target_bir_lowering=False)
    itemT = nc.declare_dram_parameter("itemT", [8, 101, NELEM], F32, isOutput=False)
    uT = nc.declare_dram_parameter("uT", [128, NB], F32, isOutput=False)
    wproj_d = nc.declare_dram_parameter("wproj", [101, 8 * 128], F32, isOutput=False)
    wt_d = nc.declare_dram_parameter("wt", [128, 128], F32, isOutput=False)
    ws_d = nc.declare_dram_parameter("ws", [128, sn * 128], F32, isOutput=False)
    ident_d = nc.declare_dram_parameter("ident", [128, 128], F32, isOutput=False)
    perm_d = nc.declare_dram_parameter("perm", [16, 8 * 128], F32, isOutput=False)
    offs_d = nc.declare_dram_parameter("offs", [128, 16], F32, isOutput=False)
    iotac_d = nc.declare_dram_parameter("iotac", [128, CORPUS], F32, isOutput=False)
    big_d = nc.declare_dram_parameter("big", [128, 1], F32, isOutput=False)
    out_d = nc.declare_dram_parameter("out", [128, sn * NB], F32, isOutput=True)
    if debug:
        dbg_corpus = nc.declare_dram_parameter("dbg_corpus", [128, NELEM], F32, isOutput=True)
        dbg_tB = nc.declare_dram_parameter("dbg_tB", [128, NELEM], F32, isOutput=True)
        dbg_sC = nc.declare_dram_parameter("dbg_sC", [128, NB], F32, isOutput=True)
        dbg_sB = nc.declare_dram_parameter("dbg_sB", [128, NB], F32, isOutput=True)
        dbg_scores = nc.declare_dram_parameter("dbg_scores", [128, 640], F32, isOutput=True)
        dbg_idxB = nc.declare_dram_parameter("dbg_idxB", [128, 64], F32, isOutput=True)
        dbg_idxs = nc.declare_dram_parameter("dbg_idxs", [128, NB // 16], I16, isOutput=True)

    with tile.TileContext(nc) as tc, ExitStack() as ctx:
        cpool = ctx.enter_context(tc.tile_pool(name="consts", bufs=1))
        cwproj = cpool.tile([101, 8 * 128], F32, tag="cwproj")
        nc.sync.dma_start(cwproj[:], wproj_d[:])
        cwt = cpool.tile([128, 128], F32, tag="cwt")
        nc.sync.dma_start(cwt[:], wt_d[:])
        cws = cpool.tile([128, sn * 128], F32, tag="cws")
        nc.sync.dma_start(cws[:], ws_d[:])
        cid = cpool.tile([128, 128], F32, tag="cid")
        nc.sync.dma_start(cid[:], ident_d[:])
        cperm = cpool.tile([16, 8 * 128], F32, tag="cperm")
        nc.sync.dma_start(cperm[:], perm_d[:])
        coffs = cpool.tile([128, 16], F32, tag="coffs")
        nc.sync.dma_start(coffs[:], offs_d[:])
        ciota = cpool.tile([128, CORPUS], F32, tag="ciota")
        nc.sync.dma_start(ciota[:], iotac_d[:])
        cbig = cpool.tile([128, 1], F32, tag="cbig")
        nc.sync.dma_start(cbig[:], big_d[:])

        # PE warm-up touches: make the PE observe each const's DMA lane via
        # tiny 1x1 matmuls so real matmuls carry at most one sync wait
        # (walrus S3_LW struct limit for fp32 matmuls).
        def pe_touch(tgt, src_ap=None):
            a = (src_ap if src_ap is not None else cid)[0:1, 0:1]
            nc.tensor.matmul(tgt[0:1, 0:1], lhsT=a, rhs=a, start=True, stop=True)
        with tc.tile_pool(name="warmps", bufs=1, space="PSUM") as wps_pool:
            wtile = wps_pool.tile([1, 1], F32, tag="wt")
            for cst in (cwproj, cwt, cid, cws, cperm):
                pe_touch(wtile, cst)

        main = ctx.enter_context(tc.tile_pool(name="main", bufs=1))
        corpus = main.tile([128, NELEM], F32, tag="corpus")
        tB = main.tile([128, NELEM], F32, tag="tB")
        outitems = main.tile([128, sn * NB], F32, tag="outitems")
        S = main.tile([128, NB], F32, tag="S")
        nc.sync.dma_start(S[:], uT[:])

        # iteration-0 s-stage depends only on S=u: emit before the projection
        sB0 = {}
        with tc.tile_pool(name="hpsum", bufs=1, space="PSUM") as hps:
            for g in range(4):
                hsps = hps.tile([128, 256], F32, tag="hsps")
                pe_touch(hsps)
                nc.tensor.matmul(hsps[:], lhsT=cws[:, 0:128],
                                 rhs=S[:, 256 * g:256 * (g + 1)],
                                 start=True, stop=True)
                hsC = main.tile([128, 256], F32, tag="hsC")
                nc.scalar.activation(hsC[:], hsps[:], _AFT.Tanh)
                hsbp = hps.tile([128, 256], F32, tag="hsbp")
                hsC_v = hsC[:].rearrange("p (q kk) -> p kk q", kk=2)
                for kl in range(2):
                    nc.tensor.transpose(hsbp[:, 128 * kl:128 * (kl + 1)],
                                        hsC_v[:, kl, :], cid[:])
                sB0g = main.tile([128, 256], F32, tag=f"hsB{g}")
                nc.scalar.activation(sB0g[:], hsbp[:], _AFT.Copy)
                sB0[g] = sB0g
        # ------- loop pools (open across the whole program) -------
        lp = ctx.enter_context(tc.tile_pool(name="loop", bufs=2))
        prp = ctx.enter_context(tc.tile_pool(name="prp", bufs=2))
        sps_pool = ctx.enter_context(tc.tile_pool(name="spsum", bufs=2, space="PSUM"))
        ips_pool = ctx.enter_context(tc.tile_pool(name="ipsum", bufs=1, space="PSUM"))

        def s_stage(ki, g):
            # s = tanh(BD(W_s/(ki+1)) @ S_g)
            sps = sps_pool.tile([128, 256], F32, tag="sps")
            pe_touch(sps)
            nc.tensor.matmul(sps[:], lhsT=cws[:, 128 * ki:128 * (ki + 1)],
                             rhs=S[:, 256 * g:256 * (g + 1)],
                             start=True, stop=True)
            sC = lp.tile([128, 256], F32, tag=f"sC{g}")
            nc.scalar.activation(sC[:], sps[:], _AFT.Tanh)
            sbp = sps_pool.tile([128, 256], F32, tag="sps")
            sC_v = sC[:].rearrange("p (q kk) -> p kk q", kk=2)
            for kl in range(2):
                nc.tensor.transpose(sbp[:, 128 * kl:128 * (kl + 1)],
                                    sC_v[:, kl, :], cid[:])
            sB = lp.tile([128, 256], F32, tag=f"sB{g}")
            nc.scalar.activation(sB[:], sbp[:], _AFT.Copy)
            return sB

        def body(ki, g, sB):
            # scores for k in {2g, 2g+1}, one chunk per k-window
            scores = lp.tile([128, 160], F32, tag=f"scores{g}")
            for kl in range(2):
                pr = prp.tile([128, 1280], F32, tag=f"pr{g % 2}")
                t_in = tB[:, (2 * g + kl) * 1280:(2 * g + kl + 1) * 1280].rearrange(
                    "p (c bh) -> p c bh", c=CORPUS)
                s_in = sB[:, 128 * kl:128 * (kl + 1)].unsqueeze(1).broadcast_to(
                    [128, CORPUS, 128])
                pr_v = pr[:].rearrange("p (c bh) -> p c bh", c=CORPUS)
                nc.vector.tensor_mul(pr_v, t_in, s_in)
                red_in = pr[:].rearrange("p (a h) -> p a h", h=HID)
                nc.vector.reduce_sum(scores[:, 80 * kl:80 * (kl + 1)],
                                     red_in, axis=mybir.AxisListType.X)
            # argmax over c (first max)
            sc_v = scores[:].rearrange("p (kk c b) -> p kk c b",
                                       kk=2, c=CORPUS).transpose([0, 1, 3, 2])
            smax = lp.tile([128, 16], F32, tag=f"smax{g}")
            smax_v = smax[:].rearrange("p (kk b) -> p kk b", kk=2)
            nc.vector.reduce_max(smax_v, sc_v, axis=mybir.AxisListType.X)
            eqm = lp.tile([128, 160], U8, tag=f"eqm{g}")
            eqm_v = eqm[:].rearrange("p (kk b c) -> p kk b c", kk=2, b=8)
            nc.vector.tensor_tensor(eqm_v, sc_v,
                                    smax_v.unsqueeze(3).broadcast_to([128, 2, 8, CORPUS]),
                                    op=mybir.AluOpType.is_equal)
            cand = lp.tile([128, 160], F32, tag=f"cand{g}")
            cand_v = cand[:].rearrange("p (kk b c) -> p kk b c", kk=2, b=8)
            iota_b = ciota[:].unsqueeze(1).unsqueeze(1).broadcast_to([128, 2, 8, CORPUS])
            big_b = cbig[:].unsqueeze(1).unsqueeze(1).broadcast_to([128, 2, 8, CORPUS])
            nc.vector.select(cand_v, eqm_v, iota_b, big_b)
            idxB = lp.tile([128, 16], F32, tag=f"idxB{g}")
            idxB_v = idxB[:].rearrange("p (kk b) -> p kk b", kk=2)
            nc.vector.tensor_reduce(idxB_v, cand_v, axis=mybir.AxisListType.X,
                                    op=mybir.AluOpType.min)
            # transport idx into ap_gather wrapped layout (itp and pps2 share
            # one PSUM slot; the WAR dep serializes them naturally)
            itp = ips_pool.tile([16, 128], F32, tag="ipsu")
            pe_touch(itp)
            nc.tensor.transpose(itp[:], idxB[:], cid[:])
            idxT = lp.tile([16, 128], F32, tag=f"idxT{g}")
            nc.scalar.activation(idxT[:], itp[:], _AFT.Copy)
            pps2 = ips_pool.tile([128, 16], F32, tag="ipsu")
            pe_touch(pps2)
            idxT_v = idxT[:].rearrange("p (s par) -> p par s", par=8)
            for par in range(8):
                nc.tensor.matmul(pps2[:], lhsT=cperm[:, 128 * par:128 * (par + 1)],
                                 rhs=idxT_v[:, par:par + 1, :],
                                 start=(par == 0), stop=(par == 7))
            idxs16 = lp.tile([128, 16], I16, tag=f"idxs16{g}")
            nc.vector.tensor_add(idxs16[:], pps2[:], coffs[:])
            # gather picks into the output slot; S update on GPSIMD (same
            # engine as the gather -> no cross-engine hop)
            iv = outitems[:, ki * NB + g * 256:ki * NB + (g + 1) * 256]
            nc.gpsimd.ap_gather(iv.rearrange("p (n d) -> p n d", d=1),
                                corpus[:, g * 2560:(g + 1) * 2560].rearrange(
                                    "p (n d) -> p n d", d=1),
                                idxs16[:], channels=128, num_elems=2560,
                                d=1, num_idxs=256)
            nc.gpsimd.tensor_tensor(S[:, 256 * g:256 * (g + 1)],
                                    S[:, 256 * g:256 * (g + 1)], iv,
                                    op=mybir.AluOpType.add)

        sB_cur = dict(sB0)

        def wave_tail(w):
            # emit loop stages whose inputs became ready during wave w-1
            for g2 in range(4):
                ki = w - 1 - g2
                if 0 <= ki < sn:
                    body(ki, g2, sB_cur[g2])
                    if ki + 1 < sn:
                        sB_cur[g2] = s_stage(ki + 1, g2)
                    if g2 == 3:
                        # iteration ki fully gathered: stream its output slot
                        nc.sync.dma_start(out_d[:, ki * NB:(ki + 1) * NB],
                                          outitems[:, ki * NB:(ki + 1) * NB])

        # ------- projection + t-phase per stripe, loop waves interleaved ----
        W = 512
        itemT_v = itemT[:].rearrange("b d n -> d b n")
        qchunks = [(0, 51), (51, 51), (102, 26)]
        with tc.tile_pool(name="proj", bufs=2) as ppool, \
             tc.tile_pool(name="ppsum", bufs=1, space="PSUM") as pps, \
             tc.tile_pool(name="tch", bufs=1) as tchp, \
             tc.tile_pool(name="tpsum", bufs=1, space="PSUM") as tps_pool, \
             tc.tile_pool(name="tbpsum", bufs=1, space="PSUM") as tbp_pool:
            for g in range(4):
                for wl in range(5):
                    w = g * 5 + wl
                    it = ppool.tile([101, 8 * W], F32, tag="it")
                    itv = it[:].rearrange("d (b n) -> d b n", b=8)
                    nc.sync.dma_start(itv, itemT_v[:, :, w * W:(w + 1) * W])
                    ps = pps.tile([128, W], F32, tag="ps")
                    pe_touch(ps)
                    for b in range(8):
                        nc.tensor.matmul(ps[:], lhsT=cwproj[:, 128 * b:128 * b + 128],
                                         rhs=it[:, b * W:(b + 1) * W],
                                         start=(b == 0), stop=(b == 7))
                    nc.scalar.activation(corpus[:, w * W:(w + 1) * W], ps[:], _AFT.Copy)
                for klw in range(2):
                    k = 2 * g + klw
                    blk = corpus[:, g * 2560:(g + 1) * 2560]
                    corpus_vk = blk.rearrange("p (q kk c) -> p kk q c", kk=2, c=CORPUS)
                    # two c-halves so PSUM tiles stay small (2 banks each)
                    for ch in range(2):
                        c0 = 5 * ch
                        tps = tps_pool.tile([128, 1024], F32, tag="tps")
                        pe_touch(tps)
                        # q-chunks of 102/26 -> 510/130 cols, bank-aligned
                        nc.tensor.matmul(tps[:, 0:510], lhsT=cwt[:],
                                         rhs=corpus_vk[:, klw, 0:102, c0:c0 + 5],
                                         start=True, stop=True)
                        nc.tensor.matmul(tps[:, 512:642], lhsT=cwt[:],
                                         rhs=corpus_vk[:, klw, 102:128, c0:c0 + 5],
                                         start=True, stop=True)
                        tch = tchp.tile([128, 640], F32, tag="tch")
                        nc.scalar.activation(tch[:, 0:510], tps[:, 0:510], _AFT.Tanh)
                        nc.scalar.activation(tch[:, 510:640], tps[:, 512:642], _AFT.Tanh)
                        tbp = tbp_pool.tile([128, 640], F32, tag="tbp")
                        tch_v = tch[:].rearrange("p (q c) -> p c q", c=5)
                        for c in range(5):
                            nc.tensor.transpose(tbp[:, 128 * c:128 * (c + 1)],
                                                tch_v[:, c, :], cid[:])
                        nc.scalar.activation(
                            tB[:, 1280 * k + 640 * ch:1280 * k + 640 * (ch + 1)],
                            tbp[:], _AFT.Copy)
                wave_tail(g + 1)

        for w in range(5, 4 + sn + 1):
            wave_tail(w)

    # move_matmul_waits_to_ldweights drops waits from self-loading fp32
    # matmuls (no standalone LDWEIGHTS exists) -> lost deps / races.
    # generate_event_semaphores alone legalizes the 1-wait-per-inst limit.
    nc.move_matmul_waits_to_ldweights = lambda: None
    if not nc.is_finalized():
        nc.finalize()
    return nc


_NC_CACHE = {}


def kernel(user_intent, item_corpus, W_proj, b_proj, W_s, W_t, search_num,
           _trace=False):
    sn = int(search_num)
    user_intent = np.asarray(user_intent, dtype=np.float32)
    item_corpus = np.asarray(item_corpus, dtype=np.float32)
    consts = build_consts(np.asarray(W_proj, dtype=np.float32),
                          np.asarray(b_proj, dtype=np.float32),
                          np.asarray(W_s, dtype=np.float32),
                          np.asarray(W_t, dtype=np.float32), sn)

    if sn not in _NC_CACHE:
        _NC_CACHE[sn] = build_nc(sn)
    nc = _NC_CACHE[sn]

    # host prep per core
    in_maps = []
    for cc in range(NCORES):
        rows = item_corpus[cc * R:(cc + 1) * R]          # [R, 10, 100]
        arr = rows.reshape(8, NB, CORPUS, IND)            # [b_lo, b_hi, c, d]
        itT = np.empty((8, 101, NELEM), dtype=np.float32)
        itT[:, :100, :] = arr.transpose(0, 3, 1, 2).reshape(8, IND, NELEM)
        itT[:, 100, :] = 1.0
        u = user_intent[cc * R:(cc + 1) * R]              # [R, 16]
        uT = u.reshape(8, NB, HID).transpose(0, 2, 1).reshape(128, NB)
        m = dict(itemT=np.ascontiguousarray(itT), uT=np.ascontiguousarray(uT))
        m.update({k: v for k, v in consts.items()})
        in_maps.append(m)

    res = run_bass_kernel_spmd(nc, in_maps, list(range(NCORES)), trace=_trace)
    if _trace:
        kernel._last_results = res

    # host post: out [128, sn*NB] layout C -> picks [R, sn, 16]
    out = np.empty((BS, 1 + sn, HID), dtype=np.float32)
    out[:, 0, :] = user_intent
    for cc in range(NCORES):
        o = res.results[cc]["out"]                        # [128, sn*NB]
        picks = o.reshape(8, HID, sn, NB).transpose(0, 3, 2, 1).reshape(R, sn, HID)
        out[cc * R:(cc + 1) * R, 1:, :] = picks
    return out
